# revision 44
# baseline (speedup 1.0000x reference)
"""Distributed Bass kernel for causal multi-head attention on 8 TRN2 NeuronCores.

Problem: B=2, S=2048, D=1024, H=16 (dh=64) causal attention layer.
Sharding: core c = (batch b = c//4, head-group g = c%4 covering 4 heads).

Communication: one 8-rank AllToAll per 512-wide q-chunk replaces the
baseline AllReduce of zero-padded partials (~4x less wire + no wasted
matmuls):
  - Output-row ownership: core d owns out^T rows [d*128,(d+1)*128) of BOTH
    batches. Each core's chunk partial (its own batch, 1024 rows x 512 q)
    is cut into 8 shards of [128, 512]; shard d goes to core d.
  - The source's batch is implied by its rank (cores 0-3 batch 0, 4-7
    batch 1), so every shard is real data. After the A2A the receiver
    tree-sums sources 0-3 into its batch-0 rows and 4-7 into batch-1
    (6 bf16 adds on vector), then writes its owned [128, 2x512] slice.
  - The host stitches the 8 owned slices; no broadcast, no AllReduce.
  - Reductions run after the whole chunk loop with an explicit dep on a
    late trigger: the tile scheduler's cost model underestimates
    collective latency and would otherwise hoist reduce work ahead of
    staging/triggers, stalling the vector/sync queues mid-compute.
  - A tiny warmup A2A absorbs first-collective overhead off the
    critical path.

Compute structure:
  - All inputs pre-cast to bf16 on the host; loads split across the
    sync+scalar HWDGE queues, states split by s-halves so QKV starts
    after half the load.
  - QKV emission is interleaved with the chunk loop so chunk 0's A2A
    fires as early as possible; chunk order [0,1,3,2] makes the final
    chunk the cheaper one.
  - Scores computed transposed [k, q]; softmax denominator rides as a
    65th V' column; no max-subtraction (scores ~N(0,1)).
  - Scores run PRE k-blocks ahead of the ctx matmuls (al ring) so the
    first ctx of a pair (which waits on a cx psum tile freed by the
    previous pair's normalization) is preceded by real tensor work --
    this keeps the PE dense, which also keeps the HAM clock gate warm.
  - Causality: k-blocks above the diagonal skipped; diagonal blocks
    column-trimmed (matmul/exp/mask/ctx restricted to cols >= m*128)
    with a multiplicative 0/1 mask.
  - Normalization: bf16 reciprocal of the [1,512] denominator rows, one
    [128,512] psum broadcast via two accumulated outer-product matmuls,
    one psum->sbuf copy, two muls into pair-stacked ctxu2.
  - ctxu2 stacks head pairs on partitions (head 2p+hi at partitions
    [hi*64,(hi+1)*64)), so the output projection contracts K=128
    (2 heads per matmul) -- half the matmuls of the K=64 variant.
  - Each chunk's output projection is deferred into the middle of the
    next chunk's attention to hide the normalization chain latency.
"""

import numpy as np
import ml_dtypes

import concourse.bass as bass
import concourse.bacc as bacc
import concourse.mybir as mybir
import concourse.tile as tile
from concourse.tile_rust import add_dep_helper
from concourse import bass_utils

DEBUG_DUMPS = False

F32 = mybir.dt.float32
BF16 = mybir.dt.bfloat16
EXP = mybir.ActivationFunctionType.Exp

B, S, D, H = 2, 2048, 1024, 16
DH = 64            # head dim
HG = 4             # heads per core (head group)
NP = 2             # head pairs per core
QC = 512           # q-chunk
NJ = S // QC       # 4 q-chunks
KB = 128           # k block (partition tile)
NKB = S // KB      # 16 k blocks
DB = D // 128      # 8 contraction blocks of 128
NCORE = 8


def _build():
    nc = bacc.Bacc(
        "TRN2", target_bir_lowering=False, debug=False,
        enable_asserts=False, num_devices=NCORE,
    )

    statesT = nc.dram_tensor("statesT", [D, S], BF16, kind="ExternalInput")
    wq_d = nc.dram_tensor("wq", [D, 256], BF16, kind="ExternalInput")
    wk_d = nc.dram_tensor("wk", [D, 256], BF16, kind="ExternalInput")
    wv_d = nc.dram_tensor("wv", [D, 256], BF16, kind="ExternalInput")
    # wo2[r, (pp*8 + ob)*128 + c] = Wo[(4g + 2pp + r//64)*64 + r%64, ob*128 + c]
    wo2_d = nc.dram_tensor("wo2", [128, NP * 8 * 128], BF16, kind="ExternalInput")
    cm_d = nc.dram_tensor("cmask", [128, 4 * QC], BF16, kind="ExternalInput")
    # out: core-owned slice; [r, b*2048 + s] = out^T[own_row_base + r, s] for batch b
    out_d = nc.dram_tensor("out", [128, B * S], BF16, kind="ExternalOutput")
    if DEBUG_DUMPS:
        dbg_qt = nc.dram_tensor("dbg_qt", [128, NP * S], BF16, kind="ExternalOutput")
        dbg_kt = nc.dram_tensor("dbg_kt", [128, NP * S], BF16, kind="ExternalOutput")
        dbg_ctxu = nc.dram_tensor("dbg_ctxu", [128, NP * S], BF16, kind="ExternalOutput")
        dbg_ccin = nc.dram_tensor("dbg_ccin", [1024, QC], BF16, kind="ExternalOutput")
        dbg_ccout = nc.dram_tensor("dbg_ccout", [1024, QC], BF16, kind="ExternalOutput")

    with tile.TileContext(nc) as tc:
        with (
            tc.tile_pool(name="const", bufs=1) as constp,
            tc.tile_pool(name="alpha", bufs=6) as alphap,
            tc.tile_pool(name="nrm", bufs=4) as nrmp,
            tc.tile_pool(name="rbs", bufs=2) as rbsp,
            tc.tile_pool(name="cxs", bufs=4) as cxsp,
            tc.tile_pool(name="stg", bufs=4) as stgp,
            tc.tile_pool(name="rb", bufs=2) as rbp,
            tc.tile_pool(name="sum", bufs=2) as sump,
            tc.tile_pool(name="ps", bufs=2, space="PSUM") as psp,
            tc.tile_pool(name="psc", bufs=2, space="PSUM") as pscp,
            tc.tile_pool(name="pso", bufs=2, space="PSUM") as psop,
            tc.tile_pool(name="dram", bufs=1, space="DRAM") as dramp,
        ):
            # ---------------- persistent SBUF tensors ----------------
            st = constp.tile([128, DB * S], BF16, tag="st")        # statesT: [d%128, db*S + s]
            wq = constp.tile([128, DB * 256], BF16, tag="wq")      # [d%128, db*256 + c]
            wk = constp.tile([128, DB * 256], BF16, tag="wk")
            wv = constp.tile([128, DB * 256], BF16, tag="wv")
            wo2 = constp.tile([128, NP * 8 * 128], BF16, tag="wo2")
            cm = constp.tile([128, 4 * QC], BF16, tag="cm")        # 4 causal mask alignments
            qt = constp.tile([128, NP * S], BF16, tag="qt")        # [pair-local c, p*S + q]
            kt = constp.tile([128, NP * S], BF16, tag="kt")
            vp = constp.tile([128, NKB * (HG * 65)], BF16, tag="vp")  # V' 65-pitch + ones col
            ctxu2 = constp.tile([128, NP * S], BF16, tag="ctxu2")  # pair-stacked normalized ctx^T
            ones2 = constp.tile([65, 256], BF16, tag="ones2")       # bcast outer-product lhsTs

            # -------- loads (all bf16; split across sync+scalar HWDGE) --------
            st_view = statesT.ap().rearrange("(a p) s -> p a s", p=128)
            nc.sync.dma_start(
                wk[:].rearrange("p (a c) -> p a c", a=DB),
                wk_d.ap().rearrange("(a p) c -> p a c", p=128))
            nc.sync.dma_start(
                wq[:].rearrange("p (a c) -> p a c", a=DB),
                wq_d.ap().rearrange("(a p) c -> p a c", p=128))
            for sh in range(2):  # s-halves: QKV for q<1024 starts after half the load
                for db in range(DB):
                    # 3-way queue split: sync/scalar HWDGE + idle gpsimd SWDGE
                    eng = (nc.sync, nc.scalar, nc.gpsimd)[db % 3]
                    eng.dma_start(
                        st[:, db * S + sh * 1024: db * S + (sh + 1) * 1024],
                        st_view[:, db, sh * 1024:(sh + 1) * 1024])
            nc.scalar.dma_start(
                wv[:].rearrange("p (a c) -> p a c", a=DB),
                wv_d.ap().rearrange("(a p) c -> p a c", p=128))
            nc.scalar.dma_start(cm[:], cm_d[:, :])
            nc.scalar.dma_start(wo2[:], wo2_d[:, :])

            # tiny warmup A2A: absorbs first-collective ncfw warmup off the
            # critical path (queues behind the NEFF-entry barrier).
            warm_in = dramp.tile([8, 128], BF16, tag="warm_in", name="warm_in")
            warm_out = dramp.tile([8, 128], BF16, tag="warm_out", name="warm_out")
            wz = stgp.tile([8, 128], BF16, tag="wz", name="wz")
            nc.vector.memset(wz[:], 0.0)
            nc.sync.dma_start(warm_in[:, :], wz[:])
            nc.gpsimd.collective_compute(
                "AllToAll", mybir.AluOpType.bypass,
                replica_groups=[list(range(NCORE))],
                ins=[warm_in[:].opt()], outs=[warm_out[:].opt()])

            nc.vector.memset(ones2[64:65, :], 0.0)
            nc.vector.memset(ones2[64:65, 0:64], 1.0)     # h0: out partitions 0-63
            nc.vector.memset(ones2[64:65, 192:256], 1.0)  # h1: out partitions 64-127
            # V' ones columns (denominator trick)
            nc.vector.memset(
                vp[:].rearrange("p (n w) -> p n w", w=65)[:, :, 64:65], 1.0)

            # ---------------- QKV projection emitters ----------------
            def emit_qk(dst, w_sb, p, jj):
                ps = psp.tile([128, 1024], F32, tag="ps",
                              name=f"qk{p}_{jj}_{dst is kt}")
                for half in range(2):
                    q0 = jj * 1024 + half * QC
                    for db in range(DB):
                        nc.tensor.matmul(
                            ps[:, half * QC:(half + 1) * QC],
                            w_sb[:, db * 256 + p * 128: db * 256 + (p + 1) * 128],
                            st[:, db * S + q0: db * S + q0 + QC],
                            start=(db == 0), stop=(db == DB - 1))
                    # copy per half so dependents start one half earlier
                    nc.any.tensor_copy(
                        dst[:, p * S + q0: p * S + q0 + QC],
                        ps[:, half * QC:(half + 1) * QC])

            def emit_v(kb):
                # V in [s, c] layout, written into 65-pitch V' slots
                psv = psop.tile([128, 512], F32, tag="pso", name=f"v{kb}")
                for db in range(DB):
                    nc.tensor.matmul(
                        psv[:, 0:256],
                        st[:, db * S + kb * KB: db * S + (kb + 1) * KB],
                        wv[:, db * 256:(db + 1) * 256],
                        start=(db == 0), stop=(db == DB - 1))
                nc.any.tensor_copy(
                    vp[:, kb * HG * 65:(kb + 1) * HG * 65]
                      .rearrange("p (h w) -> p h w", w=65)[:, :, 0:64],
                    psv[:, 0:256].rearrange("p (h w) -> p h w", w=64))

            # interleave: emit only what chunk j needs before its attention,
            # so the first A2A fires as early as possible.
            qkv_stages = {
                0: lambda: ([emit_qk(kt, wk, p, 0) for p in range(NP)],
                            [emit_qk(qt, wq, p, 0) for p in range(NP)],
                            [emit_v(kb) for kb in range(0, 4)]),
                1: lambda: ([emit_qk(kt, wk, p, 1) for p in range(NP)],
                            [emit_qk(qt, wq, p, 1) for p in range(NP)],
                            [emit_v(kb) for kb in range(4, 8)]),
                3: lambda: [emit_v(kb) for kb in range(8, 16)],
            }

            # ---- attention (j-outer; A2A per q-chunk overlaps compute) ----
            PRE = 3  # scores emitted this many kb ahead of ctx

            def emit_attn_pair(j, p):
                """Scores+ctx (kb loop) for pair p of chunk j; returns cx.
                Scores run PRE blocks ahead of ctx so the first ctx matmul
                (which waits on a cx tile freed by the previous pair's norm
                muls) is preceded by real tensor work, not a stall."""
                cx = [pscp.tile([65, QC], F32, tag="psc",
                                name=f"cx{p}_{j}_{hi}") for hi in range(2)]
                nkb = 4 * j + 4
                als = {}

                def emit_scores(kb):
                    m = kb - 4 * j
                    col0 = max(m, 0) * 128  # diagonal column trim
                    ps = psp.tile([128, 1024], F32, tag="ps",
                                  name=f"s{p}_{j}_{kb}")
                    for hi in range(2):
                        h0 = hi * 64
                        nc.tensor.matmul(
                            ps[:, hi * QC + col0:(hi + 1) * QC],
                            kt[h0:h0 + 64, p * S + kb * KB: p * S + (kb + 1) * KB],
                            qt[h0:h0 + 64, p * S + j * QC + col0: p * S + (j + 1) * QC],
                            start=True, stop=True)
                    al = alphap.tile([128, 1024], BF16, tag="alpha",
                                     name=f"al{p}_{j}_{kb}")
                    if m < 0:
                        nc.scalar.activation(al[:], ps[:], EXP, scale=0.125)
                    else:  # diagonal: trimmed exp + multiplicative causal mask
                        for hi in range(2):
                            sl = slice(hi * QC + col0, (hi + 1) * QC)
                            nc.scalar.activation(al[:, sl], ps[:, sl], EXP,
                                                 scale=0.125)
                            nc.vector.tensor_mul(
                                al[:, sl], al[:, sl],
                                cm[:, m * QC + col0:(m + 1) * QC])
                    als[kb] = al

                def emit_ctx(kb):
                    m = kb - 4 * j
                    col0 = max(m, 0) * 128
                    al = als.pop(kb)
                    for hi in range(2):
                        h = 2 * p + hi
                        nc.tensor.matmul(
                            cx[hi][:, col0:QC],
                            vp[:, kb * HG * 65 + h * 65: kb * HG * 65 + (h + 1) * 65],
                            al[:, hi * QC + col0:(hi + 1) * QC],
                            start=(kb == 0), stop=(kb == nkb - 1))

                for kb in range(nkb + PRE):
                    if kb < nkb:
                        emit_scores(kb)
                    if kb >= PRE:
                        emit_ctx(kb - PRE)
                return cx

            def emit_norm_head(j, p, cx):
                # Free the cx PSUM tiles ASAP: the only cx readers are two
                # denom recips and two ctx copies (incl. a base-shift to
                # stack the pair), so the psc ring unblocks ~1us after the
                # last ctx matmul.
                rr = nrmp.tile([65, 1024], BF16, tag="rr", name=f"rr{p}_{j}")
                with nc.allow_low_precision(
                        reason="bf16 softmax denom recip feeds bf16 bcast matmul"):
                    nc.vector.reciprocal(rr[64:65, 0:QC], cx[0][64:65, :])
                    nc.vector.reciprocal(rr[64:65, QC:2 * QC], cx[1][64:65, :])
                cxs = cxsp.tile([128, 512], F32, tag="cxs", name=f"cxs{p}_{j}")
                nc.vector.tensor_copy(cxs[0:64, :], cx[0][0:64, :])
                nc.vector.tensor_copy(cxs[64:128, :], cx[1][0:64, :])
                return rr, cxs

            def emit_norm_tail(j, p, rr, cxs):
                # Deferred with the outproj: the bcast matmuls no longer sit
                # between pairs in the tensor queue waiting on recips.
                pb = psop.tile([128, 512], F32, tag="pso", name=f"pb{p}_{j}")
                nc.tensor.matmul(pb[:], ones2[64:65, 0:128],
                                 rr[64:65, 0:QC], start=True, stop=False)
                nc.tensor.matmul(pb[:], ones2[64:65, 128:256],
                                 rr[64:65, QC:2 * QC], start=False, stop=True)
                rbs = rbsp.tile([128, 512], BF16, tag="rbs", name=f"rbs{p}_{j}")
                nc.any.tensor_copy(rbs[:], pb[:])
                for hi in range(2):
                    nc.gpsimd.tensor_mul(
                        ctxu2[hi * 64:(hi + 1) * 64,
                              p * S + j * QC: p * S + (j + 1) * QC],
                        cxs[hi * 64:(hi + 1) * 64, :],
                        rbs[hi * 64:(hi + 1) * 64, :])

            cc_pairs = []
            cc_trigs = []

            def emit_outproj(j):
                # ---- output projection for chunk j: K=128 pair matmuls ----
                # A2A shard d = this core's partial for oc block d; the
                # source's batch is implied by its rank (0-3 batch 0, 4-7
                # batch 1), so every shard is real data -- no zero padding.
                cc_in = dramp.tile([8 * 128, QC], BF16, tag=f"cci{j}", name=f"cci{j}")
                for ob in range(8):
                    po = psop.tile([128, 512], F32, tag="pso", name=f"o{j}_{ob}")
                    for pp in range(NP):
                        nc.tensor.matmul(
                            po[:],
                            wo2[:, (pp * 8 + ob) * 128:(pp * 8 + ob + 1) * 128],
                            ctxu2[:, pp * S + j * QC: pp * S + (j + 1) * QC],
                            start=(pp == 0), stop=(pp == NP - 1))
                    stage = stgp.tile([128, 512], BF16, tag="stage",
                                      name=f"stg{j}_{ob}")
                    nc.any.tensor_copy(stage[:], po[:])
                    nc.sync.dma_start(cc_in[ob * 128:(ob + 1) * 128, :], stage[:])
                if DEBUG_DUMPS and j == 0:
                    nc.scalar.dma_start(dbg_ccin[:, :], cc_in[:])
                cc_out = dramp.tile([8 * 128, QC], BF16, tag=f"cco{j}", name=f"cco{j}")
                cc = nc.gpsimd.collective_compute(
                    "AllToAll", mybir.AluOpType.bypass,
                    replica_groups=[list(range(NCORE))],
                    ins=[cc_in[:].opt()], outs=[cc_out[:].opt()])
                cc_pairs.append(cc_out)
                cc_trigs.append(cc)
                if DEBUG_DUMPS and j == 0:
                    nc.scalar.dma_start(dbg_ccout[:, :], cc_out[:])

            # Chunk order [0,1,3,2]: the cheapest remaining chunk (2) runs
            # last so the final A2A fires as early as possible. Each chunk's
            # output projection is emitted in the middle of the NEXT chunk's
            # attention: the next chunk's scores hide the norm-chain latency.
            JORDER = [0, 1, 3, 2]
            heads = {}
            for i, j in enumerate(JORDER):
                if j in qkv_stages:
                    qkv_stages[j]()
                for p in range(NP):
                    cx = emit_attn_pair(j, p)
                    if p == 0 and i > 0:
                        jp = JORDER[i - 1]
                        for p2 in range(NP):
                            emit_norm_tail(jp, p2, *heads.pop((jp, p2)))
                        emit_outproj(jp)
                    heads[(j, p)] = emit_norm_head(j, p, cx)
            jl = JORDER[-1]
            for p2 in range(NP):
                emit_norm_tail(jl, p2, *heads.pop((jl, p2)))
            emit_outproj(jl)
            # All reductions after the loop: all A2As except the last one
            # completed long ago. The explicit dep on a late trigger keeps
            # the tile scheduler (whose cost model underestimates collective
            # latency) from hoisting reduce work ahead of staging/triggers.
            for i, j in enumerate(JORDER):
                _reduce_chunk(nc, rbp, sump, cc_pairs[i], out_d, j,
                              after=cc_trigs[min(i + 2, NJ - 1)])
            if DEBUG_DUMPS:
                nc.scalar.dma_start(dbg_qt[:, :], qt[:])
                nc.scalar.dma_start(dbg_kt[:, :], kt[:])
                nc.scalar.dma_start(dbg_ctxu[:, :], ctxu2[:])

    nc.compile()
    return nc


def _reduce_chunk(nc, rbp, sump, cc_out, out_d, j, after=None):
    """Read back the A2A result for chunk j (sync HWDGE), tree-sum the 4
    sources per batch (srcs 0-3 are batch 0, 4-7 batch 1) on vector in
    bf16, and write this core's owned [128, 2x512] output slice. Each
    batch half is read/summed/written independently so the second half's
    DMA overlaps the first half's adds."""
    rb = rbp.tile([128, 8 * QC], BF16, tag="rb", name=f"rb{j}")
    rbv = rb[:].rearrange("p (s w) -> p s w", s=8)
    ccv = cc_out[:].rearrange("(s p) w -> p s w", p=128)
    rb4 = rb[:].rearrange("p (s x) -> p s x", s=8)
    out_v = out_d.ap().rearrange("p (h ss) -> p h ss", h=2)
    acc = sump.tile([128, 2 * QC], BF16, tag="acc", name=f"acc{j}")
    # both readback halves first: the second DMA must not sit behind the
    # first half's output write (which waits on vector adds) in the sync FIFO
    for h in range(2):
        rb_dma = nc.sync.dma_start(rbv[:, 4 * h:4 * h + 4, :],
                                   ccv[:, 4 * h:4 * h + 4, :])
        if after is not None:
            add_dep_helper(rb_dma.ins, after.ins, sync=False,
                           reason="keep reduces after all staging/triggers")
    for h in range(2):
        t0 = sump.tile([128, QC], BF16, tag="t0", name=f"t0_{j}_{h}")
        t1 = sump.tile([128, QC], BF16, tag="t1", name=f"t1_{j}_{h}")
        with nc.allow_low_precision(reason="bf16 tree-sum of A2A shards"):
            nc.vector.tensor_add(t0[:], rb4[:, 4 * h + 0, :], rb4[:, 4 * h + 1, :])
            nc.vector.tensor_add(t1[:], rb4[:, 4 * h + 2, :], rb4[:, 4 * h + 3, :])
            nc.vector.tensor_add(acc[:, h * QC:(h + 1) * QC], t0[:], t1[:])
        nc.sync.dma_start(out_v[:, h, j * QC:(j + 1) * QC],
                          acc[:, h * QC:(h + 1) * QC])


_NC = None
_LAST_RESULTS = None  # BassKernelResults of the most recent run (for test harness)


def _causal_mask_tiles() -> np.ndarray:
    r = np.arange(128)[:, None]
    col = np.arange(QC)[None, :]
    tiles = [(col >= r + 128 * m).astype(np.float32) for m in range(4)]
    return np.concatenate(tiles, axis=1)  # [128, 2048]


def _wo2_input(Wo: np.ndarray, g: int) -> np.ndarray:
    """Pair-packed Wo stationary slices: [128, (pp*8 + ob)*128 + c] with
    wo2[r, ...] = Wo[(4g + 2pp + r//64)*64 + r%64, ob*128 + c]."""
    wo2 = np.empty((128, NP * 8 * 128), np.float32)
    for pp in range(NP):
        for half in range(2):
            h = 4 * g + 2 * pp + half
            blk = Wo[h * DH:(h + 1) * DH, :]  # [64, 1024]
            for ob in range(8):
                wo2[half * 64:(half + 1) * 64, (pp * 8 + ob) * 128:
                    (pp * 8 + ob + 1) * 128] = blk[:, ob * 128:(ob + 1) * 128]
    return wo2


def kernel(states, masks, Wq, Wk, Wv, Wo):
    global _NC, _LAST_RESULTS
    if _NC is None:
        _NC = _build()
    bf16 = ml_dtypes.bfloat16
    states = np.asarray(states, np.float32)
    Wq, Wk, Wv, Wo = (np.asarray(w, np.float32) for w in (Wq, Wk, Wv, Wo))
    cm = _causal_mask_tiles().astype(bf16)

    in_maps = []
    for c in range(NCORE):
        b, g = c // 4, c % 4
        cs = slice(g * 256, (g + 1) * 256)
        in_maps.append({
            "statesT": np.ascontiguousarray(states[b].T).astype(bf16),
            "wq": np.ascontiguousarray(Wq[:, cs]).astype(bf16),
            "wk": np.ascontiguousarray(Wk[:, cs]).astype(bf16),
            "wv": np.ascontiguousarray(Wv[:, cs]).astype(bf16),
            "wo2": _wo2_input(Wo, g).astype(bf16),
            "cmask": cm,
        })

    res = bass_utils.run_bass_kernel_spmd(_NC, in_maps, core_ids=list(range(NCORE)))
    _LAST_RESULTS = res
    # core c owns out^T rows [c*128,(c+1)*128) of each batch; stitch + transpose
    out = np.empty((B, S, D), np.float32)
    for c in range(NCORE):
        blk = np.asarray(res.results[c]["out"]).astype(np.float32)  # [128, 2*2048]
        for b in range(B):
            out[b][:, c * 128:(c + 1) * 128] = blk[:, b * S:(b + 1) * S].T
    return out


# revision 45
# speedup vs baseline: 1.1037x; 1.1037x over previous
"""Distributed Bass kernel for causal multi-head attention on 8 TRN2 NeuronCores.

Problem: B=2, S=2048, D=1024, H=16 (dh=64) causal attention layer.
Sharding: core c = (batch b = c//4, head-group g = c%4 covering 4 heads).

Communication: one 8-rank AllToAll per 512-wide q-chunk replaces the
baseline AllReduce of zero-padded partials (~4x less wire + no wasted
matmuls):
  - Output-row ownership: core d owns out^T rows [d*128,(d+1)*128) of BOTH
    batches. Each core's chunk partial (its own batch, 1024 rows x 512 q)
    is cut into 8 shards of [128, 512]; shard d goes to core d.
  - The source's batch is implied by its rank (cores 0-3 batch 0, 4-7
    batch 1), so every shard is real data. After the A2A the receiver
    tree-sums sources 0-3 into its batch-0 rows and 4-7 into batch-1
    (6 bf16 adds on vector), then writes its owned [128, 2x512] slice.
  - The host stitches the 8 owned slices; no broadcast, no AllReduce.
  - Reductions run after the whole chunk loop with an explicit dep on a
    late trigger: the tile scheduler's cost model underestimates
    collective latency and would otherwise hoist reduce work ahead of
    staging/triggers, stalling the vector/sync queues mid-compute.
  - A tiny warmup A2A absorbs first-collective overhead off the
    critical path.

Compute structure:
  - All inputs pre-cast to bf16 on the host; loads split across the
    sync+scalar HWDGE queues, states split by s-halves so QKV starts
    after half the load.
  - QKV emission is interleaved with the chunk loop so chunk 0's A2A
    fires as early as possible; chunk order [0,1,3,2] makes the final
    chunk the cheaper one.
  - Scores computed transposed [k, q]; softmax denominator rides as a
    65th V' column; no max-subtraction (scores ~N(0,1)).
  - Scores run PRE k-blocks ahead of the ctx matmuls (al ring) so the
    first ctx of a pair (which waits on a cx psum tile freed by the
    previous pair's normalization) is preceded by real tensor work --
    this keeps the PE dense, which also keeps the HAM clock gate warm.
  - Causality: k-blocks above the diagonal skipped; diagonal blocks
    column-trimmed (matmul/exp/mask/ctx restricted to cols >= m*128)
    with a multiplicative 0/1 mask.
  - Normalization: bf16 reciprocal of the [1,512] denominator rows, one
    [128,512] psum broadcast via two accumulated outer-product matmuls,
    one psum->sbuf copy, two muls into pair-stacked ctxu2.
  - ctxu2 stacks head pairs on partitions (head 2p+hi at partitions
    [hi*64,(hi+1)*64)), so the output projection contracts K=128
    (2 heads per matmul) -- half the matmuls of the K=64 variant.
  - Each chunk's output projection is deferred into the middle of the
    next chunk's attention to hide the normalization chain latency.
"""

import numpy as np
import ml_dtypes

import concourse.bass as bass
import concourse.bacc as bacc
import concourse.mybir as mybir
import concourse.tile as tile
from concourse.tile_rust import add_dep_helper
from concourse import bass_utils

DEBUG_DUMPS = False

F32 = mybir.dt.float32
BF16 = mybir.dt.bfloat16
EXP = mybir.ActivationFunctionType.Exp

B, S, D, H = 2, 2048, 1024, 16
DH = 64            # head dim
HG = 4             # heads per core (head group)
NP = 2             # head pairs per core
QC = 512           # q-chunk
NJ = S // QC       # 4 q-chunks
KB = 128           # k block (partition tile)
NKB = S // KB      # 16 k blocks
DB = D // 128      # 8 contraction blocks of 128
NCORE = 8


def _build():
    nc = bacc.Bacc(
        "TRN2", target_bir_lowering=False, debug=False,
        enable_asserts=False, num_devices=NCORE,
    )

    statesT = nc.dram_tensor("statesT", [D, S], BF16, kind="ExternalInput")
    wq_d = nc.dram_tensor("wq", [D, 256], BF16, kind="ExternalInput")
    wk_d = nc.dram_tensor("wk", [D, 256], BF16, kind="ExternalInput")
    wv_d = nc.dram_tensor("wv", [D, 256], BF16, kind="ExternalInput")
    # wo2[r, (pp*8 + ob)*128 + c] = Wo[(4g + 2pp + r//64)*64 + r%64, ob*128 + c]
    wo2_d = nc.dram_tensor("wo2", [128, NP * 8 * 128], BF16, kind="ExternalInput")
    cm_d = nc.dram_tensor("cmask", [128, 4 * QC], BF16, kind="ExternalInput")
    # out: core-owned slice; [r, b*2048 + s] = out^T[own_row_base + r, s] for batch b
    out_d = nc.dram_tensor("out", [128, B * S], BF16, kind="ExternalOutput")
    if DEBUG_DUMPS:
        dbg_qt = nc.dram_tensor("dbg_qt", [128, NP * S], BF16, kind="ExternalOutput")
        dbg_kt = nc.dram_tensor("dbg_kt", [128, NP * S], BF16, kind="ExternalOutput")
        dbg_ctxu = nc.dram_tensor("dbg_ctxu", [128, NP * S], BF16, kind="ExternalOutput")
        dbg_ccin = nc.dram_tensor("dbg_ccin", [1024, QC], BF16, kind="ExternalOutput")
        dbg_ccout = nc.dram_tensor("dbg_ccout", [1024, QC], BF16, kind="ExternalOutput")

    with tile.TileContext(nc) as tc:
        with (
            tc.tile_pool(name="const", bufs=1) as constp,
            tc.tile_pool(name="alpha", bufs=6) as alphap,
            tc.tile_pool(name="nrm", bufs=4) as nrmp,
            tc.tile_pool(name="rbs", bufs=2) as rbsp,
            tc.tile_pool(name="cxs", bufs=4) as cxsp,
            tc.tile_pool(name="stg", bufs=4) as stgp,
            tc.tile_pool(name="rb", bufs=2) as rbp,
            tc.tile_pool(name="sum", bufs=2) as sump,
            tc.tile_pool(name="ps", bufs=2, space="PSUM") as psp,
            tc.tile_pool(name="psc", bufs=2, space="PSUM") as pscp,
            tc.tile_pool(name="pso", bufs=2, space="PSUM") as psop,
            tc.tile_pool(name="dram", bufs=1, space="DRAM") as dramp,
        ):
            # ---------------- persistent SBUF tensors ----------------
            st = constp.tile([128, DB * S], BF16, tag="st")        # statesT: [d%128, db*S + s]
            wq = constp.tile([128, DB * 256], BF16, tag="wq")      # [d%128, db*256 + c]
            wk = constp.tile([128, DB * 256], BF16, tag="wk")
            wv = constp.tile([128, DB * 256], BF16, tag="wv")
            wo2 = constp.tile([128, NP * 8 * 128], BF16, tag="wo2")
            cm = constp.tile([128, 4 * QC], BF16, tag="cm")        # 4 causal mask alignments
            qt = constp.tile([128, NP * S], BF16, tag="qt")        # [pair-local c, p*S + q]
            kt = constp.tile([128, NP * S], BF16, tag="kt")
            vp = constp.tile([128, NKB * (HG * 65)], BF16, tag="vp")  # V' 65-pitch + ones col
            ctxu2 = constp.tile([128, NP * S], BF16, tag="ctxu2")  # pair-stacked normalized ctx^T
            ones2 = constp.tile([65, 256], BF16, tag="ones2")       # bcast outer-product lhsTs

            # -------- loads (all bf16; split across sync+scalar HWDGE) --------
            st_view = statesT.ap().rearrange("(a p) s -> p a s", p=128)
            nc.sync.dma_start(
                wk[:].rearrange("p (a c) -> p a c", a=DB),
                wk_d.ap().rearrange("(a p) c -> p a c", p=128))
            nc.sync.dma_start(
                wq[:].rearrange("p (a c) -> p a c", a=DB),
                wq_d.ap().rearrange("(a p) c -> p a c", p=128))
            for sh in range(2):  # s-halves: QKV for q<1024 starts after half the load
                for db in range(DB):
                    eng = nc.sync if db % 2 == 0 else nc.scalar
                    eng.dma_start(
                        st[:, db * S + sh * 1024: db * S + (sh + 1) * 1024],
                        st_view[:, db, sh * 1024:(sh + 1) * 1024])
            nc.scalar.dma_start(
                wv[:].rearrange("p (a c) -> p a c", a=DB),
                wv_d.ap().rearrange("(a p) c -> p a c", p=128))
            nc.scalar.dma_start(cm[:], cm_d[:, :])
            nc.scalar.dma_start(wo2[:], wo2_d[:, :])

            # tiny warmup A2A: absorbs first-collective ncfw warmup off the
            # critical path (queues behind the NEFF-entry barrier).
            warm_in = dramp.tile([8, 128], BF16, tag="warm_in", name="warm_in")
            warm_out = dramp.tile([8, 128], BF16, tag="warm_out", name="warm_out")
            wz = stgp.tile([8, 128], BF16, tag="wz", name="wz")
            nc.vector.memset(wz[:], 0.0)
            nc.sync.dma_start(warm_in[:, :], wz[:])
            nc.gpsimd.collective_compute(
                "AllToAll", mybir.AluOpType.bypass,
                replica_groups=[list(range(NCORE))],
                ins=[warm_in[:].opt()], outs=[warm_out[:].opt()])

            nc.vector.memset(ones2[64:65, :], 0.0)
            nc.vector.memset(ones2[64:65, 0:64], 1.0)     # h0: out partitions 0-63
            nc.vector.memset(ones2[64:65, 192:256], 1.0)  # h1: out partitions 64-127
            # V' ones columns (denominator trick)
            nc.vector.memset(
                vp[:].rearrange("p (n w) -> p n w", w=65)[:, :, 64:65], 1.0)

            # ---------------- QKV projection emitters ----------------
            def emit_qk(dst, w_sb, p, jj):
                ps = psp.tile([128, 1024], F32, tag="ps",
                              name=f"qk{p}_{jj}_{dst is kt}")
                for half in range(2):
                    q0 = jj * 1024 + half * QC
                    for db in range(DB):
                        nc.tensor.matmul(
                            ps[:, half * QC:(half + 1) * QC],
                            w_sb[:, db * 256 + p * 128: db * 256 + (p + 1) * 128],
                            st[:, db * S + q0: db * S + q0 + QC],
                            start=(db == 0), stop=(db == DB - 1))
                    # copy per half so dependents start one half earlier
                    nc.any.tensor_copy(
                        dst[:, p * S + q0: p * S + q0 + QC],
                        ps[:, half * QC:(half + 1) * QC])

            def emit_v(kb):
                # V in [s, c] layout, written into 65-pitch V' slots
                psv = psop.tile([128, 512], F32, tag="pso", name=f"v{kb}")
                for db in range(DB):
                    nc.tensor.matmul(
                        psv[:, 0:256],
                        st[:, db * S + kb * KB: db * S + (kb + 1) * KB],
                        wv[:, db * 256:(db + 1) * 256],
                        start=(db == 0), stop=(db == DB - 1))
                nc.any.tensor_copy(
                    vp[:, kb * HG * 65:(kb + 1) * HG * 65]
                      .rearrange("p (h w) -> p h w", w=65)[:, :, 0:64],
                    psv[:, 0:256].rearrange("p (h w) -> p h w", w=64))

            # interleave: emit only what chunk j needs before its attention,
            # so the first A2A fires as early as possible.
            qkv_stages = {
                0: lambda: ([emit_qk(kt, wk, p, 0) for p in range(NP)],
                            [emit_qk(qt, wq, p, 0) for p in range(NP)],
                            [emit_v(kb) for kb in range(0, 4)]),
                1: lambda: ([emit_qk(kt, wk, p, 1) for p in range(NP)],
                            [emit_qk(qt, wq, p, 1) for p in range(NP)],
                            [emit_v(kb) for kb in range(4, 8)]),
                3: lambda: [emit_v(kb) for kb in range(8, 16)],
            }

            # ---- attention (j-outer; A2A per q-chunk overlaps compute) ----
            PRE = 3  # scores emitted this many kb ahead of ctx

            def emit_attn_pair(j, p):
                """Scores+ctx (kb loop) for pair p of chunk j; returns cx.
                Scores run PRE blocks ahead of ctx so the first ctx matmul
                (which waits on a cx tile freed by the previous pair's norm
                muls) is preceded by real tensor work, not a stall."""
                cx = [pscp.tile([65, QC], F32, tag="psc",
                                name=f"cx{p}_{j}_{hi}") for hi in range(2)]
                nkb = 4 * j + 4
                als = {}

                def emit_scores(kb):
                    m = kb - 4 * j
                    col0 = max(m, 0) * 128  # diagonal column trim
                    ps = psp.tile([128, 1024], F32, tag="ps",
                                  name=f"s{p}_{j}_{kb}")
                    for hi in range(2):
                        h0 = hi * 64
                        nc.tensor.matmul(
                            ps[:, hi * QC + col0:(hi + 1) * QC],
                            kt[h0:h0 + 64, p * S + kb * KB: p * S + (kb + 1) * KB],
                            qt[h0:h0 + 64, p * S + j * QC + col0: p * S + (j + 1) * QC],
                            start=True, stop=True)
                    al = alphap.tile([128, 1024], BF16, tag="alpha",
                                     name=f"al{p}_{j}_{kb}")
                    if m < 0:
                        nc.scalar.activation(al[:], ps[:], EXP, scale=0.125)
                    else:  # diagonal: trimmed exp + multiplicative causal mask
                        for hi in range(2):
                            sl = slice(hi * QC + col0, (hi + 1) * QC)
                            nc.scalar.activation(al[:, sl], ps[:, sl], EXP,
                                                 scale=0.125)
                            nc.vector.tensor_mul(
                                al[:, sl], al[:, sl],
                                cm[:, m * QC + col0:(m + 1) * QC])
                    als[kb] = al

                def emit_ctx(kb):
                    m = kb - 4 * j
                    col0 = max(m, 0) * 128
                    al = als.pop(kb)
                    for hi in range(2):
                        h = 2 * p + hi
                        nc.tensor.matmul(
                            cx[hi][:, col0:QC],
                            vp[:, kb * HG * 65 + h * 65: kb * HG * 65 + (h + 1) * 65],
                            al[:, hi * QC + col0:(hi + 1) * QC],
                            start=(kb == 0), stop=(kb == nkb - 1))

                for kb in range(nkb + PRE):
                    if kb < nkb:
                        emit_scores(kb)
                    if kb >= PRE:
                        emit_ctx(kb - PRE)
                return cx

            def emit_norm_head(j, p, cx):
                # Free the cx PSUM tiles ASAP: the only cx readers are two
                # denom recips and two ctx copies (incl. a base-shift to
                # stack the pair), so the psc ring unblocks ~1us after the
                # last ctx matmul.
                rr = nrmp.tile([65, 1024], BF16, tag="rr", name=f"rr{p}_{j}")
                with nc.allow_low_precision(
                        reason="bf16 softmax denom recip feeds bf16 bcast matmul"):
                    nc.vector.reciprocal(rr[64:65, 0:QC], cx[0][64:65, :])
                    nc.vector.reciprocal(rr[64:65, QC:2 * QC], cx[1][64:65, :])
                cxs = cxsp.tile([128, 512], F32, tag="cxs", name=f"cxs{p}_{j}")
                nc.vector.tensor_copy(cxs[0:64, :], cx[0][0:64, :])
                nc.vector.tensor_copy(cxs[64:128, :], cx[1][0:64, :])
                return rr, cxs

            def emit_norm_tail(j, p, rr, cxs):
                # Deferred with the outproj: the bcast matmuls no longer sit
                # between pairs in the tensor queue waiting on recips.
                pb = psop.tile([128, 512], F32, tag="pso", name=f"pb{p}_{j}")
                nc.tensor.matmul(pb[:], ones2[64:65, 0:128],
                                 rr[64:65, 0:QC], start=True, stop=False)
                nc.tensor.matmul(pb[:], ones2[64:65, 128:256],
                                 rr[64:65, QC:2 * QC], start=False, stop=True)
                rbs = rbsp.tile([128, 512], BF16, tag="rbs", name=f"rbs{p}_{j}")
                nc.any.tensor_copy(rbs[:], pb[:])
                for hi in range(2):
                    nc.gpsimd.tensor_mul(
                        ctxu2[hi * 64:(hi + 1) * 64,
                              p * S + j * QC: p * S + (j + 1) * QC],
                        cxs[hi * 64:(hi + 1) * 64, :],
                        rbs[hi * 64:(hi + 1) * 64, :])

            cc_pairs = []
            cc_trigs = []

            def emit_outproj(j):
                # ---- output projection for chunk j: K=128 pair matmuls ----
                # A2A shard d = this core's partial for oc block d; the
                # source's batch is implied by its rank (0-3 batch 0, 4-7
                # batch 1), so every shard is real data -- no zero padding.
                cc_in = dramp.tile([8 * 128, QC], BF16, tag=f"cci{j}", name=f"cci{j}")
                for ob in range(8):
                    po = psop.tile([128, 512], F32, tag="pso", name=f"o{j}_{ob}")
                    for pp in range(NP):
                        nc.tensor.matmul(
                            po[:],
                            wo2[:, (pp * 8 + ob) * 128:(pp * 8 + ob + 1) * 128],
                            ctxu2[:, pp * S + j * QC: pp * S + (j + 1) * QC],
                            start=(pp == 0), stop=(pp == NP - 1))
                    stage = stgp.tile([128, 512], BF16, tag="stage",
                                      name=f"stg{j}_{ob}")
                    nc.any.tensor_copy(stage[:], po[:])
                    nc.sync.dma_start(cc_in[ob * 128:(ob + 1) * 128, :], stage[:])
                if DEBUG_DUMPS and j == 0:
                    nc.scalar.dma_start(dbg_ccin[:, :], cc_in[:])
                cc_out = dramp.tile([8 * 128, QC], BF16, tag=f"cco{j}", name=f"cco{j}")
                cc = nc.gpsimd.collective_compute(
                    "AllToAll", mybir.AluOpType.bypass,
                    replica_groups=[list(range(NCORE))],
                    ins=[cc_in[:].opt()], outs=[cc_out[:].opt()])
                cc_pairs.append(cc_out)
                cc_trigs.append(cc)
                if DEBUG_DUMPS and j == 0:
                    nc.scalar.dma_start(dbg_ccout[:, :], cc_out[:])

            # Chunk order [0,1,3,2]: the cheapest remaining chunk (2) runs
            # last so the final A2A fires as early as possible. Each chunk's
            # output projection is emitted in the middle of the NEXT chunk's
            # attention: the next chunk's scores hide the norm-chain latency.
            JORDER = [0, 1, 3, 2]
            heads = {}
            for i, j in enumerate(JORDER):
                if j in qkv_stages:
                    qkv_stages[j]()
                for p in range(NP):
                    cx = emit_attn_pair(j, p)
                    if p == 0 and i > 0:
                        jp = JORDER[i - 1]
                        for p2 in range(NP):
                            emit_norm_tail(jp, p2, *heads.pop((jp, p2)))
                        emit_outproj(jp)
                    heads[(j, p)] = emit_norm_head(j, p, cx)
            jl = JORDER[-1]
            for p2 in range(NP):
                emit_norm_tail(jl, p2, *heads.pop((jl, p2)))
            emit_outproj(jl)
            # All reductions after the loop: all A2As except the last one
            # completed long ago. The explicit dep on a late trigger keeps
            # the tile scheduler (whose cost model underestimates collective
            # latency) from hoisting reduce work ahead of staging/triggers.
            for i, j in enumerate(JORDER):
                _reduce_chunk(nc, rbp, sump, cc_pairs[i], out_d, j,
                              after=cc_trigs[min(i + 2, NJ - 1)])
            if DEBUG_DUMPS:
                nc.scalar.dma_start(dbg_qt[:, :], qt[:])
                nc.scalar.dma_start(dbg_kt[:, :], kt[:])
                nc.scalar.dma_start(dbg_ctxu[:, :], ctxu2[:])

    nc.compile()
    return nc


def _reduce_chunk(nc, rbp, sump, cc_out, out_d, j, after=None):
    """Read back the A2A result for chunk j (sync HWDGE), tree-sum the 4
    sources per batch (srcs 0-3 are batch 0, 4-7 batch 1) on vector in
    bf16, and write this core's owned [128, 2x512] output slice. Each
    batch half is read/summed/written independently so the second half's
    DMA overlaps the first half's adds."""
    rb = rbp.tile([128, 8 * QC], BF16, tag="rb", name=f"rb{j}")
    rbv = rb[:].rearrange("p (s w) -> p s w", s=8)
    ccv = cc_out[:].rearrange("(s p) w -> p s w", p=128)
    rb4 = rb[:].rearrange("p (s x) -> p s x", s=8)
    out_v = out_d.ap().rearrange("p (h ss) -> p h ss", h=2)
    acc = sump.tile([128, 2 * QC], BF16, tag="acc", name=f"acc{j}")
    # both readback halves first: the second DMA must not sit behind the
    # first half's output write (which waits on vector adds) in the sync FIFO
    for h in range(2):
        rb_dma = nc.sync.dma_start(rbv[:, 4 * h:4 * h + 4, :],
                                   ccv[:, 4 * h:4 * h + 4, :])
        if after is not None:
            add_dep_helper(rb_dma.ins, after.ins, sync=False,
                           reason="keep reduces after all staging/triggers")
    for h in range(2):
        t0 = sump.tile([128, QC], BF16, tag="t0", name=f"t0_{j}_{h}")
        t1 = sump.tile([128, QC], BF16, tag="t1", name=f"t1_{j}_{h}")
        with nc.allow_low_precision(reason="bf16 tree-sum of A2A shards"):
            nc.vector.tensor_add(t0[:], rb4[:, 4 * h + 0, :], rb4[:, 4 * h + 1, :])
            nc.vector.tensor_add(t1[:], rb4[:, 4 * h + 2, :], rb4[:, 4 * h + 3, :])
            nc.vector.tensor_add(acc[:, h * QC:(h + 1) * QC], t0[:], t1[:])
        nc.sync.dma_start(out_v[:, h, j * QC:(j + 1) * QC],
                          acc[:, h * QC:(h + 1) * QC])


_NC = None
_LAST_RESULTS = None  # BassKernelResults of the most recent run (for test harness)


def _causal_mask_tiles() -> np.ndarray:
    r = np.arange(128)[:, None]
    col = np.arange(QC)[None, :]
    tiles = [(col >= r + 128 * m).astype(np.float32) for m in range(4)]
    return np.concatenate(tiles, axis=1)  # [128, 2048]


def _wo2_input(Wo: np.ndarray, g: int) -> np.ndarray:
    """Pair-packed Wo stationary slices: [128, (pp*8 + ob)*128 + c] with
    wo2[r, ...] = Wo[(4g + 2pp + r//64)*64 + r%64, ob*128 + c]."""
    wo2 = np.empty((128, NP * 8 * 128), np.float32)
    for pp in range(NP):
        for half in range(2):
            h = 4 * g + 2 * pp + half
            blk = Wo[h * DH:(h + 1) * DH, :]  # [64, 1024]
            for ob in range(8):
                wo2[half * 64:(half + 1) * 64, (pp * 8 + ob) * 128:
                    (pp * 8 + ob + 1) * 128] = blk[:, ob * 128:(ob + 1) * 128]
    return wo2


def kernel(states, masks, Wq, Wk, Wv, Wo):
    global _NC, _LAST_RESULTS
    if _NC is None:
        _NC = _build()
    bf16 = ml_dtypes.bfloat16
    states = np.asarray(states, np.float32)
    Wq, Wk, Wv, Wo = (np.asarray(w, np.float32) for w in (Wq, Wk, Wv, Wo))
    cm = _causal_mask_tiles().astype(bf16)

    in_maps = []
    for c in range(NCORE):
        b, g = c // 4, c % 4
        cs = slice(g * 256, (g + 1) * 256)
        in_maps.append({
            "statesT": np.ascontiguousarray(states[b].T).astype(bf16),
            "wq": np.ascontiguousarray(Wq[:, cs]).astype(bf16),
            "wk": np.ascontiguousarray(Wk[:, cs]).astype(bf16),
            "wv": np.ascontiguousarray(Wv[:, cs]).astype(bf16),
            "wo2": _wo2_input(Wo, g).astype(bf16),
            "cmask": cm,
        })

    res = bass_utils.run_bass_kernel_spmd(_NC, in_maps, core_ids=list(range(NCORE)))
    _LAST_RESULTS = res
    # core c owns out^T rows [c*128,(c+1)*128) of each batch; stitch + transpose
    out = np.empty((B, S, D), np.float32)
    for c in range(NCORE):
        blk = np.asarray(res.results[c]["out"]).astype(np.float32)  # [128, 2*2048]
        for b in range(B):
            out[b][:, c * 128:(c + 1) * 128] = blk[:, b * S:(b + 1) * S].T
    return out
